# revision 6
# baseline (speedup 1.0000x reference)
"""Trainium2 Bass kernel for nn_Attention_13537736917778.

Full inputs -> full output. Sharding: 8 cores = 2 (batch) x 4 (head groups of 4).
Per-core: channel-major flash attention (S^T layout, keys on partitions).
Softmax denominators are produced replicated across 64 PSUM rows by ones-columns
in the PV stationary operand; normalization and RMS-norm reciprocals both run as
ACT ln/exp pairs (exp table set only), so no DVE reciprocal and no PE broadcast
matmuls. Out-projection partial sums are reduced on host.
"""
import sys
import numpy as np

sys.path.insert(0, "/opt/trn_rl_repo")

import ml_dtypes
import concourse.bass as bass
import concourse.mybir as mybir
from concourse import tile
from concourse.bass_utils import run_bass_kernel_spmd
from contextlib import ExitStack

bf16 = mybir.dt.bfloat16
f32 = mybir.dt.float32

B, N, C = 2, 2048, 1024
H, D = 16, 64
G = 4              # heads per core
NT = N             # tokens per core (one batch)
FT = 512
TI = NT // FT      # 4 i-tiles
KC = C // 128      # 8 input-channel chunks
JC = NT // 128     # 16 key chunks
OC = 3 * G * D // 128   # 6 qkv output chunks
EPS = 1e-6
SCALE = 1.0 / 8.0  # 1/sqrt(64)

_CACHE = {}


def _split_waits(nc, limit=1):
    """walrus CTRL has one hw wait slot; split multi-wait instructions into
    NOP chains carrying the extra waits."""
    counter = 0
    for fn in nc.m.functions:
        for bb in fn.blocks:
            new_insts = []
            for inst in bb.instructions:
                si = inst.sync_info
                if si is not None and si.on_wait and len(si.on_wait) > limit:
                    waits = list(si.on_wait)
                    head, tail = waits[:-limit], waits[-limit:]
                    for i in range(0, len(head), limit):
                        nop = mybir.InstNoOp(
                            name=f"I-waitsplit-{counter}", ins=[], outs=[]
                        )
                        counter += 1
                        nop.engine = inst.engine
                        nop.sync_info = mybir.SyncInfo(
                            on_wait=head[i : i + limit], on_update=[]
                        )
                        new_insts.append(nop)
                    inst.sync_info = mybir.SyncInfo(
                        on_wait=tail, on_update=list(si.on_update)
                    )
                new_insts.append(inst)
            bb.instructions[:] = new_insts
    return counter


def _build_nc():
    nc = bass.Bass()
    xT = nc.declare_dram_parameter("xT", [C, NT], bf16, isOutput=False)
    wqkvT = nc.declare_dram_parameter("wqkvT", [C, OC * 128], bf16, isOutput=False)
    bqkv = nc.declare_dram_parameter("bqkv", [128, OC], f32, isOutput=False)
    wrep = nc.declare_dram_parameter("wrep", [128, 2], f32, isOutput=False)
    iden = nc.declare_dram_parameter("iden", [128, 128], bf16, isOutput=False)
    woT = nc.declare_dram_parameter("woT", [2, 128, C], bf16, isOutput=False)
    y = nc.declare_dram_parameter("y", [NT, C], f32, isOutput=True)

    xT_r = xT.rearrange("(kc p) n -> kc p n", p=128)
    wqkvT_r = wqkvT.rearrange("(kc p) o -> kc p o", p=128)

    Exp = mybir.ActivationFunctionType.Exp
    Ln = mybir.ActivationFunctionType.Ln
    MUL = mybir.AluOpType.mult

    with tile.TileContext(nc) as tc:
        with ExitStack() as ctx:
            perm = ctx.enter_context(tc.tile_pool(name="perm", bufs=1))
            # ---- persistent tiles ----
            iden_sb = perm.tile([128, 128], bf16, name="iden_sb", tag="iden_sb")
            nc.sync.dma_start(out=iden_sb[:], in_=iden[:])
            bias_sb = perm.tile([128, OC + 1], f32, name="bias_sb", tag="bias_sb")
            nc.sync.dma_start(out=bias_sb[:, 0:OC], in_=bqkv[:])
            nc.vector.memset(bias_sb[:, OC : OC + 1], EPS)
            wrep_sb = perm.tile([128, 2], f32, name="wrep_sb", tag="wrep_sb")
            nc.sync.dma_start(out=wrep_sb[:], in_=wrep[:])
            ones_mask = perm.tile([128, 128], bf16, name="ones_mask", tag="ones_mask")
            nc.vector.memset(ones_mask[:], 0.0)
            nc.vector.memset(ones_mask[0:64, 0:64], 1.0)
            nc.vector.memset(ones_mask[64:128, 64:128], 1.0)

            w_sb = []
            for kc in range(KC):
                t = perm.tile([128, OC * 128], bf16, name=f"w_sb{kc}", tag=f"w_sb{kc}")
                nc.sync.dma_start(out=t[:], in_=wqkvT_r[kc])
                w_sb.append(t)
            woT_sb = []
            for oo in range(2):
                t = perm.tile([128, C], bf16, name=f"woT_sb{oo}", tag=f"woT_sb{oo}")
                nc.sync.dma_start(out=t[:], in_=woT[oo])
                woT_sb.append(t)

            qk_sb = [
                perm.tile([128, NT], f32, name=f"qk_sb{i}", tag=f"qk_sb{i}")
                for i in range(4)
            ]
            v16_sb = [
                perm.tile([128, NT], bf16, name=f"v16_sb{i}", tag=f"v16_sb{i}")
                for i in range(2)
            ]
            qhat = [
                perm.tile([128, NT], bf16, name=f"qhat{i}", tag=f"qhat{i}")
                for i in range(2)
            ]
            khat = [
                perm.tile([128, NT], bf16, name=f"khat{i}", tag=f"khat{i}")
                for i in range(2)
            ]
            # per jc: [A-V(64) | A-ones(64) | B-V(64) | B-ones(64)]
            vtok = [
                perm.tile([128, JC, 256], bf16, name=f"vtok{i}", tag=f"vtok{i}")
                for i in range(2)
            ]
            onT = [
                perm.tile([128, NT], bf16, name=f"onT{i}", tag=f"onT{i}")
                for i in range(2)
            ]
            rr_bc = [
                perm.tile([128, NT], f32, name=f"rr_bc{i}", tag=f"rr_bc{i}")
                for i in range(4)
            ]
            for c in range(2):
                nc.vector.memset(vtok[c][:, :, 64:128], 1.0)
                nc.vector.memset(vtok[c][:, :, 192:256], 1.0)

            # ---- phase A/B/D pools ----
            with ExitStack() as pctx:
                xpool = pctx.enter_context(tc.tile_pool(name="xpool", bufs=12))
                sqpool = pctx.enter_context(tc.tile_pool(name="sqpool", bufs=2))
                t1pool = pctx.enter_context(tc.tile_pool(name="t1pool", bufs=1))
                pqkv = pctx.enter_context(
                    tc.tile_pool(name="pqkv", bufs=2, space="PSUM")
                )
                mspool = pctx.enter_context(
                    tc.tile_pool(name="mspool", bufs=1, space="PSUM")
                )
                auxp = pctx.enter_context(
                    tc.tile_pool(name="auxp", bufs=2, space="PSUM")
                )

                # Phase A: QKV projection (channel-major output)
                for it in range(TI):
                    isl = slice(it * FT, (it + 1) * FT)
                    xt = []
                    for kc in range(KC):
                        t = xpool.tile([128, FT], bf16, name=f"xt{it}_{kc}", tag="xt")
                        nc.sync.dma_start(out=t[:], in_=xT_r[kc][:, isl])
                        xt.append(t)
                    for oc in range(OC):
                        ps = pqkv.tile([128, FT], f32, name=f"pqkv{it}_{oc}", tag="pqkv")
                        for kc in range(KC):
                            nc.tensor.matmul(
                                ps[:],
                                lhsT=w_sb[kc][:, oc * 128 : (oc + 1) * 128],
                                rhs=xt[kc][:],
                                start=(kc == 0),
                                stop=(kc == KC - 1),
                            )
                        dst = qk_sb[oc] if oc < 4 else v16_sb[oc - 4]
                        nc.vector.tensor_scalar_add(
                            dst[:, isl], ps[:], bias_sb[:, oc : oc + 1]
                        )

                # Phase D: V transposes -> token-major vtok
                for c in range(2):
                    for jc in range(JC):
                        pt = auxp.tile([128, 128], bf16, name=f"vt{c}_{jc}", tag="aux")
                        nc.tensor.transpose(
                            pt[:], v16_sb[c][:, jc * 128 : (jc + 1) * 128], iden_sb[:]
                        )
                        nc.vector.tensor_copy(vtok[c][:, jc, 0:64], pt[:, 0:64])
                        nc.vector.tensor_copy(vtok[c][:, jc, 128:192], pt[:, 64:128])

                # Phase B: RMS norm factors (replicated) + qhat/khat
                for qc in range(2):
                    for qk_i, (src, rr, wcol, dst) in enumerate(
                        [
                            (qk_sb[qc], rr_bc[qc], 0, qhat[qc]),
                            (qk_sb[2 + qc], rr_bc[2 + qc], 1, khat[qc]),
                        ]
                    ):
                        sq = sqpool.tile([128, NT], bf16, name=f"sq{qc}_{qk_i}", tag="sq")
                        nc.vector.tensor_mul(sq[:], src[:], src[:])
                        ms = mspool.tile([128, NT], f32, name=f"ms{qc}_{qk_i}", tag="ms")
                        for t in range(TI):
                            tsl = slice(t * FT, (t + 1) * FT)
                            nc.tensor.matmul(
                                ms[:, tsl],
                                lhsT=ones_mask[:],
                                rhs=sq[:, tsl],
                                start=True,
                                stop=True,
                            )
                        t1 = t1pool.tile([128, NT], f32, name=f"t1_{qc}_{qk_i}", tag="t1")
                        nc.scalar.activation(
                            t1[:], ms[:], Ln, scale=1.0 / D, bias=bias_sb[:, OC : OC + 1]
                        )
                        nc.scalar.activation(rr[:], t1[:], Exp, scale=-0.5)
                        nc.vector.scalar_tensor_tensor(
                            dst[:],
                            src[:],
                            wrep_sb[:, wcol : wcol + 1],
                            rr[:],
                            MUL,
                            MUL,
                        )

            # ---- attention + out-projection pools ----
            with ExitStack() as pctx:
                s4pool = pctx.enter_context(
                    tc.tile_pool(name="s4pool", bufs=1, space="PSUM")
                )
                pvpool = pctx.enter_context(
                    tc.tile_pool(name="pvpool", bufs=2, space="PSUM")
                )
                ptpool = pctx.enter_context(tc.tile_pool(name="ptpool", bufs=3))
                dnpool = pctx.enter_context(tc.tile_pool(name="dnpool", bufs=2))
                ypool = pctx.enter_context(tc.tile_pool(name="ypool", bufs=3))

                for it in range(TI):
                    isl = slice(it * FT, (it + 1) * FT)
                    for qc in range(2):
                        pv2 = pvpool.tile(
                            [128, 2 * FT], f32, name=f"pv2_{qc}_{it}", tag="pv"
                        )
                        for jcp in range(JC // 2):
                            j0 = 2 * jcp
                            j1 = 2 * jcp + 1
                            j0sl = slice(j0 * 128, (j0 + 1) * 128)
                            j1sl = slice(j1 * 128, (j1 + 1) * 128)
                            s4 = s4pool.tile(
                                [128, 4 * FT], f32, name=f"s4_{qc}_{it}_{jcp}", tag="s4"
                            )
                            nc.tensor.matmul(
                                s4[:, 0:FT],
                                lhsT=khat[qc][0:64, j0sl],
                                rhs=qhat[qc][0:64, isl],
                                start=True,
                                stop=True,
                            )
                            nc.tensor.matmul(
                                s4[:, FT : 2 * FT],
                                lhsT=khat[qc][64:128, j0sl],
                                rhs=qhat[qc][64:128, isl],
                                start=True,
                                stop=True,
                            )
                            nc.tensor.matmul(
                                s4[:, 2 * FT : 3 * FT],
                                lhsT=khat[qc][0:64, j1sl],
                                rhs=qhat[qc][0:64, isl],
                                start=True,
                                stop=True,
                            )
                            nc.tensor.matmul(
                                s4[:, 3 * FT : 4 * FT],
                                lhsT=khat[qc][64:128, j1sl],
                                rhs=qhat[qc][64:128, isl],
                                start=True,
                                stop=True,
                            )
                            pt = ptpool.tile(
                                [128, 4 * FT], bf16, name=f"pt{qc}_{it}_{jcp}", tag="pt"
                            )
                            nc.scalar.activation(pt[:], s4[:], Exp, scale=SCALE)
                            nc.tensor.matmul(
                                pv2[:, 0:FT],
                                lhsT=vtok[qc][:, j0, 0:128],
                                rhs=pt[:, 0:FT],
                                start=(jcp == 0),
                                stop=False,
                            )
                            nc.tensor.matmul(
                                pv2[:, FT : 2 * FT],
                                lhsT=vtok[qc][:, j0, 128:256],
                                rhs=pt[:, FT : 2 * FT],
                                start=(jcp == 0),
                                stop=False,
                            )
                            nc.tensor.matmul(
                                pv2[:, 0:FT],
                                lhsT=vtok[qc][:, j1, 0:128],
                                rhs=pt[:, 2 * FT : 3 * FT],
                                start=False,
                                stop=(jcp == JC // 2 - 1),
                            )
                            nc.tensor.matmul(
                                pv2[:, FT : 2 * FT],
                                lhsT=vtok[qc][:, j1, 128:256],
                                rhs=pt[:, 3 * FT : 4 * FT],
                                start=False,
                                stop=(jcp == JC // 2 - 1),
                            )
                        # normalize: O = PV * exp(-ln(denom)); denom replicated rows 64:128
                        td = dnpool.tile([64, 2 * FT], f32, name=f"td{qc}_{it}", tag="td")
                        nc.scalar.activation(td[:], pv2[64:128, :], Ln)
                        bcr = dnpool.tile(
                            [64, 2 * FT], f32, name=f"bcr{qc}_{it}", tag="bcr"
                        )
                        nc.scalar.activation(bcr[:], td[:], Exp, scale=-1.0)
                        nc.vector.tensor_mul(
                            onT[qc][0:64, isl], pv2[0:64, 0:FT], bcr[:, 0:FT]
                        )
                        nc.vector.tensor_mul(
                            onT[qc][64:128, isl],
                            pv2[0:64, FT : 2 * FT],
                            bcr[:, FT : 2 * FT],
                        )

                    # out-projection for this it's 4 token chunks
                    for ic in range(it * 4, it * 4 + 4):
                        csl = slice(ic * 128, (ic + 1) * 128)
                        p01 = pvpool.tile([128, 2 * FT], f32, name=f"p01_{ic}", tag="pv")
                        for oo in range(2):
                            nc.tensor.matmul(
                                p01[:, 0:FT],
                                lhsT=onT[oo][:, csl],
                                rhs=woT_sb[oo][:, 0:FT],
                                start=(oo == 0),
                                stop=(oo == 1),
                            )
                            nc.tensor.matmul(
                                p01[:, FT : 2 * FT],
                                lhsT=onT[oo][:, csl],
                                rhs=woT_sb[oo][:, FT : 2 * FT],
                                start=(oo == 0),
                                stop=(oo == 1),
                            )
                        yt = ypool.tile([128, C], f32, name=f"yt{ic}", tag="yt")
                        nc.vector.tensor_copy(yt[:], p01[:])
                        nc.sync.dma_start(out=y[csl, :], in_=yt[:])

    _split_waits(nc, limit=1)
    return nc


def _prep_inputs(x, Wq, bq, Wk, bk, Wv, bv, q_norm_w, k_norm_w, Wo, bo):
    bf = ml_dtypes.bfloat16
    x = np.asarray(x, dtype=np.float32)
    Wfull = np.concatenate(
        [np.asarray(Wq), np.asarray(Wk), np.asarray(Wv)], axis=0
    ).astype(np.float32)
    bfull = np.concatenate(
        [np.asarray(bq), np.asarray(bk), np.asarray(bv)], axis=0
    ).astype(np.float32)
    Wo = np.asarray(Wo, dtype=np.float32)
    q_norm_w = np.asarray(q_norm_w, dtype=np.float32)
    k_norm_w = np.asarray(k_norm_w, dtype=np.float32)

    xT_b = [np.ascontiguousarray(x[b].T).astype(bf) for b in range(B)]
    IDEN = np.eye(128, dtype=np.float32).astype(bf)
    wrep = np.stack(
        [np.tile(q_norm_w, 2), np.tile(k_norm_w, 2)], axis=1
    ).astype(np.float32)

    in_maps = []
    for core in range(8):
        b = core // 4
        hg = core % 4
        heads = [hg * 4 + i for i in range(G)]
        q_rows = np.concatenate([Wfull[192 * h : 192 * h + 64] for h in heads], axis=0)
        k_rows = np.concatenate(
            [Wfull[192 * h + 64 : 192 * h + 128] for h in heads], axis=0
        )
        v_rows = np.concatenate(
            [Wfull[192 * h + 128 : 192 * h + 192] for h in heads], axis=0
        )
        W_shard = np.concatenate([q_rows, k_rows, v_rows], axis=0)  # [768, 1024]
        bq_rows = np.concatenate([bfull[192 * h : 192 * h + 64] for h in heads])
        bk_rows = np.concatenate([bfull[192 * h + 64 : 192 * h + 128] for h in heads])
        bv_rows = np.concatenate([bfull[192 * h + 128 : 192 * h + 192] for h in heads])
        b_shard = np.concatenate([bq_rows, bk_rows, bv_rows])  # [768]
        cols = np.concatenate([np.arange(64 * h, 64 * h + 64) for h in heads])
        WoT_shard = np.ascontiguousarray(Wo[:, cols].T)  # [256, 1024]

        in_maps.append(
            {
                "xT": xT_b[b],
                "wqkvT": np.ascontiguousarray(W_shard.T).astype(bf),
                "bqkv": np.ascontiguousarray(b_shard.reshape(OC, 128).T).astype(
                    np.float32
                ),
                "wrep": wrep,
                "iden": IDEN,
                "woT": WoT_shard.reshape(2, 128, C).astype(bf),
            }
        )
    return in_maps


def kernel(**inputs):
    if "nc" not in _CACHE:
        _CACHE["nc"] = _build_nc()
    nc = _CACHE["nc"]
    in_maps = _prep_inputs(**inputs)
    res = run_bass_kernel_spmd(nc, in_maps, list(range(8)))
    bo = np.asarray(inputs["bo"], dtype=np.float32)
    y = np.zeros((B, N, C), dtype=np.float32)
    for core in range(8):
        y[core // 4] += res.results[core]["y"]
    y += bo[None, None, :]
    return y


# revision 7
# speedup vs baseline: 1.2219x; 1.2219x over previous
"""Trainium2 Bass kernel for nn_Attention_13537736917778.

Full inputs -> full output. Sharding: 8 cores = 2 (batch) x 4 (head groups of 4).
Per-core: channel-major flash attention (S^T layout, keys on partitions).
Softmax denominators are produced replicated across 64 PSUM rows by ones-columns
in the PV stationary operand; normalization and RMS-norm reciprocals both run as
ACT ln/exp pairs (exp table set only), so no DVE reciprocal and no PE broadcast
matmuls. Out-projection partial sums are reduced on host.
"""
import sys
import numpy as np

sys.path.insert(0, "/opt/trn_rl_repo")

import ml_dtypes
import concourse.bass as bass
import concourse.mybir as mybir
from concourse import tile
from concourse.bass_utils import run_bass_kernel_spmd
from contextlib import ExitStack

bf16 = mybir.dt.bfloat16
f32 = mybir.dt.float32

B, N, C = 2, 2048, 1024
H, D = 16, 64
G = 4              # heads per core
NT = N             # tokens per core (one batch)
FT = 512
TI = NT // FT      # 4 i-tiles
KC = C // 128      # 8 input-channel chunks
JC = NT // 128     # 16 key chunks
OC = 3 * G * D // 128   # 6 qkv output chunks
EPS = 1e-6
SCALE = 1.0 / 8.0  # 1/sqrt(64)

_CACHE = {}


def _split_waits(nc, limit=1):
    """walrus CTRL has one hw wait slot; split multi-wait instructions into
    NOP chains carrying the extra waits."""
    counter = 0
    for fn in nc.m.functions:
        for bb in fn.blocks:
            new_insts = []
            for inst in bb.instructions:
                si = inst.sync_info
                if si is not None and si.on_wait and len(si.on_wait) > limit:
                    waits = list(si.on_wait)
                    head, tail = waits[:-limit], waits[-limit:]
                    for i in range(0, len(head), limit):
                        nop = mybir.InstNoOp(
                            name=f"I-waitsplit-{counter}", ins=[], outs=[]
                        )
                        counter += 1
                        nop.engine = inst.engine
                        nop.sync_info = mybir.SyncInfo(
                            on_wait=head[i : i + limit], on_update=[]
                        )
                        new_insts.append(nop)
                    inst.sync_info = mybir.SyncInfo(
                        on_wait=tail, on_update=list(si.on_update)
                    )
                new_insts.append(inst)
            bb.instructions[:] = new_insts
    return counter


def _build_nc():
    nc = bass.Bass()
    xT = nc.declare_dram_parameter("xT", [C, NT], bf16, isOutput=False)
    wqkvT = nc.declare_dram_parameter("wqkvT", [C, OC * 128], bf16, isOutput=False)
    bqkv = nc.declare_dram_parameter("bqkv", [128, OC], f32, isOutput=False)
    wrep = nc.declare_dram_parameter("wrep", [128, 2], f32, isOutput=False)
    iden = nc.declare_dram_parameter("iden", [128, 128], bf16, isOutput=False)
    woT = nc.declare_dram_parameter("woT", [2, 128, C], bf16, isOutput=False)
    y = nc.declare_dram_parameter("y", [NT, C], f32, isOutput=True)

    xT_r = xT.rearrange("(kc p) n -> kc p n", p=128)
    wqkvT_r = wqkvT.rearrange("(kc p) o -> kc p o", p=128)

    Exp = mybir.ActivationFunctionType.Exp
    Ln = mybir.ActivationFunctionType.Ln
    MUL = mybir.AluOpType.mult

    with tile.TileContext(nc) as tc:
        with ExitStack() as ctx:
            perm = ctx.enter_context(tc.tile_pool(name="perm", bufs=1))
            # ---- persistent tiles ----
            iden_sb = perm.tile([128, 128], bf16, name="iden_sb", tag="iden_sb")
            nc.sync.dma_start(out=iden_sb[:], in_=iden[:])
            bias_sb = perm.tile([128, OC + 1], f32, name="bias_sb", tag="bias_sb")
            nc.sync.dma_start(out=bias_sb[:, 0:OC], in_=bqkv[:])
            nc.vector.memset(bias_sb[:, OC : OC + 1], EPS)
            wrep_sb = perm.tile([128, 2], f32, name="wrep_sb", tag="wrep_sb")
            nc.sync.dma_start(out=wrep_sb[:], in_=wrep[:])
            ones_mask = perm.tile([128, 128], bf16, name="ones_mask", tag="ones_mask")
            nc.vector.memset(ones_mask[:], 0.0)
            nc.vector.memset(ones_mask[0:64, 0:64], 1.0)
            nc.vector.memset(ones_mask[64:128, 64:128], 1.0)

            w_sb = []
            for kc in range(KC):
                t = perm.tile([128, OC * 128], bf16, name=f"w_sb{kc}", tag=f"w_sb{kc}")
                nc.sync.dma_start(out=t[:], in_=wqkvT_r[kc])
                w_sb.append(t)
            woT_sb = []
            for oo in range(2):
                t = perm.tile([128, C], bf16, name=f"woT_sb{oo}", tag=f"woT_sb{oo}")
                nc.sync.dma_start(out=t[:], in_=woT[oo])
                woT_sb.append(t)

            qk_sb = [
                perm.tile([128, NT], f32, name=f"qk_sb{i}", tag=f"qk_sb{i}")
                for i in range(4)
            ]
            v16_sb = [
                perm.tile([128, NT], bf16, name=f"v16_sb{i}", tag=f"v16_sb{i}")
                for i in range(2)
            ]
            qhat = [
                perm.tile([128, NT], bf16, name=f"qhat{i}", tag=f"qhat{i}")
                for i in range(2)
            ]
            khat = [
                perm.tile([128, NT], bf16, name=f"khat{i}", tag=f"khat{i}")
                for i in range(2)
            ]
            # per jc: [A-V(64) | A-ones(64) | B-V(64) | B-ones(64)]
            vtok = [
                perm.tile([128, JC, 256], bf16, name=f"vtok{i}", tag=f"vtok{i}")
                for i in range(2)
            ]
            onT = [
                perm.tile([128, NT], bf16, name=f"onT{i}", tag=f"onT{i}")
                for i in range(2)
            ]
            rr_bc = [
                perm.tile([128, NT], f32, name=f"rr_bc{i}", tag=f"rr_bc{i}")
                for i in range(4)
            ]
            for c in range(2):
                nc.vector.memset(vtok[c][:, :, 64:128], 1.0)
                nc.vector.memset(vtok[c][:, :, 192:256], 1.0)

            # ---- phase A/B/D pools ----
            with ExitStack() as pctx:
                xpool = pctx.enter_context(tc.tile_pool(name="xpool", bufs=12))
                sqpool = pctx.enter_context(tc.tile_pool(name="sqpool", bufs=2))
                t1pool = pctx.enter_context(tc.tile_pool(name="t1pool", bufs=1))
                pqkv = pctx.enter_context(
                    tc.tile_pool(name="pqkv", bufs=2, space="PSUM")
                )
                mspool = pctx.enter_context(
                    tc.tile_pool(name="mspool", bufs=1, space="PSUM")
                )
                auxp = pctx.enter_context(
                    tc.tile_pool(name="auxp", bufs=2, space="PSUM")
                )

                # Phase A: QKV projection (channel-major output)
                for it in range(TI):
                    isl = slice(it * FT, (it + 1) * FT)
                    xt = []
                    for kc in range(KC):
                        t = xpool.tile([128, FT], bf16, name=f"xt{it}_{kc}", tag="xt")
                        nc.sync.dma_start(out=t[:], in_=xT_r[kc][:, isl])
                        xt.append(t)
                    for oc in range(OC):
                        ps = pqkv.tile([128, FT], f32, name=f"pqkv{it}_{oc}", tag="pqkv")
                        for kc in range(KC):
                            nc.tensor.matmul(
                                ps[:],
                                lhsT=w_sb[kc][:, oc * 128 : (oc + 1) * 128],
                                rhs=xt[kc][:],
                                start=(kc == 0),
                                stop=(kc == KC - 1),
                            )
                        dst = qk_sb[oc] if oc < 4 else v16_sb[oc - 4]
                        nc.vector.tensor_scalar_add(
                            dst[:, isl], ps[:], bias_sb[:, oc : oc + 1]
                        )

                # Phase D: V transposes -> token-major vtok
                for c in range(2):
                    for jc in range(JC):
                        pt = auxp.tile([128, 128], bf16, name=f"vt{c}_{jc}", tag="aux")
                        nc.tensor.transpose(
                            pt[:], v16_sb[c][:, jc * 128 : (jc + 1) * 128], iden_sb[:]
                        )
                        nc.vector.tensor_copy(vtok[c][:, jc, 0:64], pt[:, 0:64])
                        nc.vector.tensor_copy(vtok[c][:, jc, 128:192], pt[:, 64:128])

                # Phase B: RMS norm factors (replicated) + qhat/khat
                for qc in range(2):
                    for qk_i, (src, rr, wcol, dst) in enumerate(
                        [
                            (qk_sb[qc], rr_bc[qc], 0, qhat[qc]),
                            (qk_sb[2 + qc], rr_bc[2 + qc], 1, khat[qc]),
                        ]
                    ):
                        sq = sqpool.tile([128, NT], bf16, name=f"sq{qc}_{qk_i}", tag="sq")
                        nc.vector.tensor_mul(sq[:], src[:], src[:])
                        ms = mspool.tile([128, NT], f32, name=f"ms{qc}_{qk_i}", tag="ms")
                        for t in range(TI):
                            tsl = slice(t * FT, (t + 1) * FT)
                            nc.tensor.matmul(
                                ms[:, tsl],
                                lhsT=ones_mask[:],
                                rhs=sq[:, tsl],
                                start=True,
                                stop=True,
                            )
                        t1 = t1pool.tile([128, NT], f32, name=f"t1_{qc}_{qk_i}", tag="t1")
                        nc.scalar.activation(
                            t1[:], ms[:], Ln, scale=1.0 / D, bias=bias_sb[:, OC : OC + 1]
                        )
                        nc.scalar.activation(rr[:], t1[:], Exp, scale=-0.5)
                        nc.vector.scalar_tensor_tensor(
                            dst[:],
                            src[:],
                            wrep_sb[:, wcol : wcol + 1],
                            rr[:],
                            MUL,
                            MUL,
                        )

            # ---- attention + out-projection pools ----
            with ExitStack() as pctx:
                s4pool = pctx.enter_context(
                    tc.tile_pool(name="s4pool", bufs=2, space="PSUM")
                )
                pvpool = pctx.enter_context(
                    tc.tile_pool(name="pvpool", bufs=1, space="PSUM")
                )
                ptpool = pctx.enter_context(tc.tile_pool(name="ptpool", bufs=3))
                dnpool = pctx.enter_context(tc.tile_pool(name="dnpool", bufs=2))
                ypool = pctx.enter_context(tc.tile_pool(name="ypool", bufs=3))

                for it in range(TI):
                    isl = slice(it * FT, (it + 1) * FT)
                    for qc in range(2):
                        pv2 = pvpool.tile(
                            [128, 2 * FT], f32, name=f"pv2_{qc}_{it}", tag="pv"
                        )
                        # 32 (head, jc) S-blocks in sequence, packed 3 per PSUM tile
                        NSEQ = 2 * JC  # 32
                        PER = 3
                        s3 = None
                        pt3 = None
                        for s in range(NSEQ):
                            head = s % 2
                            jc = s // 2
                            jsl = slice(jc * 128, (jc + 1) * 128)
                            slot = s % PER
                            if slot == 0:
                                s3 = s4pool.tile(
                                    [128, PER * FT], f32, name=f"s3_{qc}_{it}_{s}", tag="s4"
                                )
                            ssl = slice(slot * FT, (slot + 1) * FT)
                            if head == 0:
                                nc.tensor.matmul(
                                    s3[:, ssl],
                                    lhsT=khat[qc][0:64, jsl],
                                    rhs=qhat[qc][0:64, isl],
                                    start=True,
                                    stop=True,
                                )
                            else:
                                nc.tensor.matmul(
                                    s3[:, ssl],
                                    lhsT=khat[qc][64:128, jsl],
                                    rhs=qhat[qc][64:128, isl],
                                    start=True,
                                    stop=True,
                                )
                            if slot == PER - 1 or s == NSEQ - 1:
                                nfull = slot + 1
                                pt3 = ptpool.tile(
                                    [128, PER * FT], bf16, name=f"pt{qc}_{it}_{s}", tag="pt"
                                )
                                nc.scalar.activation(
                                    pt3[:, 0 : nfull * FT],
                                    s3[:, 0 : nfull * FT],
                                    Exp,
                                    scale=SCALE,
                                )
                                for sb in range(s - nfull + 1, s + 1):
                                    hb = sb % 2
                                    jb = sb // 2
                                    bsl = slice((sb % PER) * FT, (sb % PER + 1) * FT)
                                    nc.tensor.matmul(
                                        pv2[:, hb * FT : (hb + 1) * FT],
                                        lhsT=vtok[qc][:, jb, hb * 128 : (hb + 1) * 128],
                                        rhs=pt3[:, bsl],
                                        start=(sb == hb),
                                        stop=(sb >= NSEQ - 2),
                                    )
                        # normalize: O = PV * exp(-ln(denom)); denom replicated rows 64:128
                        td = dnpool.tile([64, 2 * FT], f32, name=f"td{qc}_{it}", tag="td")
                        nc.scalar.activation(td[:], pv2[64:128, :], Ln)
                        bcr = dnpool.tile(
                            [64, 2 * FT], f32, name=f"bcr{qc}_{it}", tag="bcr"
                        )
                        nc.scalar.activation(bcr[:], td[:], Exp, scale=-1.0)
                        nc.vector.tensor_mul(
                            onT[qc][0:64, isl], pv2[0:64, 0:FT], bcr[:, 0:FT]
                        )
                        nc.vector.tensor_mul(
                            onT[qc][64:128, isl],
                            pv2[0:64, FT : 2 * FT],
                            bcr[:, FT : 2 * FT],
                        )

                    # out-projection for this it's 4 token chunks
                    for ic in range(it * 4, it * 4 + 4):
                        csl = slice(ic * 128, (ic + 1) * 128)
                        p01 = pvpool.tile([128, 2 * FT], f32, name=f"p01_{ic}", tag="pv")
                        for oo in range(2):
                            nc.tensor.matmul(
                                p01[:, 0:FT],
                                lhsT=onT[oo][:, csl],
                                rhs=woT_sb[oo][:, 0:FT],
                                start=(oo == 0),
                                stop=(oo == 1),
                            )
                            nc.tensor.matmul(
                                p01[:, FT : 2 * FT],
                                lhsT=onT[oo][:, csl],
                                rhs=woT_sb[oo][:, FT : 2 * FT],
                                start=(oo == 0),
                                stop=(oo == 1),
                            )
                        yt = ypool.tile([128, C], f32, name=f"yt{ic}", tag="yt")
                        nc.vector.tensor_copy(yt[:], p01[:])
                        nc.sync.dma_start(out=y[csl, :], in_=yt[:])

    _split_waits(nc, limit=1)
    return nc


def _prep_inputs(x, Wq, bq, Wk, bk, Wv, bv, q_norm_w, k_norm_w, Wo, bo):
    bf = ml_dtypes.bfloat16
    x = np.asarray(x, dtype=np.float32)
    Wfull = np.concatenate(
        [np.asarray(Wq), np.asarray(Wk), np.asarray(Wv)], axis=0
    ).astype(np.float32)
    bfull = np.concatenate(
        [np.asarray(bq), np.asarray(bk), np.asarray(bv)], axis=0
    ).astype(np.float32)
    Wo = np.asarray(Wo, dtype=np.float32)
    q_norm_w = np.asarray(q_norm_w, dtype=np.float32)
    k_norm_w = np.asarray(k_norm_w, dtype=np.float32)

    xT_b = [np.ascontiguousarray(x[b].T).astype(bf) for b in range(B)]
    IDEN = np.eye(128, dtype=np.float32).astype(bf)
    wrep = np.stack(
        [np.tile(q_norm_w, 2), np.tile(k_norm_w, 2)], axis=1
    ).astype(np.float32)

    in_maps = []
    for core in range(8):
        b = core // 4
        hg = core % 4
        heads = [hg * 4 + i for i in range(G)]
        q_rows = np.concatenate([Wfull[192 * h : 192 * h + 64] for h in heads], axis=0)
        k_rows = np.concatenate(
            [Wfull[192 * h + 64 : 192 * h + 128] for h in heads], axis=0
        )
        v_rows = np.concatenate(
            [Wfull[192 * h + 128 : 192 * h + 192] for h in heads], axis=0
        )
        W_shard = np.concatenate([q_rows, k_rows, v_rows], axis=0)  # [768, 1024]
        bq_rows = np.concatenate([bfull[192 * h : 192 * h + 64] for h in heads])
        bk_rows = np.concatenate([bfull[192 * h + 64 : 192 * h + 128] for h in heads])
        bv_rows = np.concatenate([bfull[192 * h + 128 : 192 * h + 192] for h in heads])
        b_shard = np.concatenate([bq_rows, bk_rows, bv_rows])  # [768]
        cols = np.concatenate([np.arange(64 * h, 64 * h + 64) for h in heads])
        WoT_shard = np.ascontiguousarray(Wo[:, cols].T)  # [256, 1024]

        in_maps.append(
            {
                "xT": xT_b[b],
                "wqkvT": np.ascontiguousarray(W_shard.T).astype(bf),
                "bqkv": np.ascontiguousarray(b_shard.reshape(OC, 128).T).astype(
                    np.float32
                ),
                "wrep": wrep,
                "iden": IDEN,
                "woT": WoT_shard.reshape(2, 128, C).astype(bf),
            }
        )
    return in_maps


def kernel(**inputs):
    if "nc" not in _CACHE:
        _CACHE["nc"] = _build_nc()
    nc = _CACHE["nc"]
    in_maps = _prep_inputs(**inputs)
    res = run_bass_kernel_spmd(nc, in_maps, list(range(8)))
    bo = np.asarray(inputs["bo"], dtype=np.float32)
    y = np.zeros((B, N, C), dtype=np.float32)
    for core in range(8):
        y[core // 4] += res.results[core]["y"]
    y += bo[None, None, :]
    return y
